# revision 2
# baseline (speedup 1.0000x reference)
"""Trainium2 Bass kernel for nn_Encoder (dense transformer encoder layer).

Sharding: 8 NeuronCores, sequence-parallel. B*S = 2*2048 = 4096 rows ->
512 rows per core; cores 0-3 handle batch 0, cores 4-7 batch 1. Each core
computes Q/K/V for its own rows, AllGathers K^T and V within its 4-core
batch group, then runs attention for its rows over all 16 heads, plus
Wo / LN1 / FFN / LN2 locally (no further communication).

Dataflow is feature-on-partition ("transposed") for all matmuls, f32r
dtype (full PE rate, ~1e-4 relative error):
  x [512,1024] --PE transpose--> xT [1024,512]
  QT/KT per head-pair [128, 512] = Wq_slice.T @ xT
  V natural [512, 1024] = xT_tile.T @ Wv_slice
  AllGather KT -> [4096, 512] and V -> [2048, 1024] per batch group
  S^T tile [128 keys, 512 q] = KT_slice.T @ QT_head (2-head row packing)
  P^T = exp(0.125 * S^T)   (ACT f32-out, then DVE round-copy to f32r)
  O^T [64,512]  += V_slice.T @ P^T   (2-head column packing)
  den^T [1,512] += ones.T @ P^T      (column offsets 0/32)
  oT = O^T * broadcast(1/den) + bv   (PE broadcast of 1/den)
  mhaT = Wo_slice.T @ oT (+bo) -> PE transpose -> +x -> LN1 -> x1n (f32r)
  x1n --PE transpose--> x1nT;  hT = relu(W1.T @ x1nT + b1)
  ffnT = W2.T @ hT + b2 -> PE transpose -> +x1n -> LN2 -> out [512,1024]
"""

import numpy as np

import concourse.bass as bass
import concourse.mybir as mybir
from concourse import bacc
from concourse.tile import TileContext
from concourse.bass_utils import run_bass_kernel_spmd

F32 = mybir.dt.float32
F32R = mybir.dt.float32r
AF = mybir.ActivationFunctionType
OP = mybir.AluOpType
AX = mybir.AxisListType

B, S, D = 2, 2048, 1024
H, DK, DFF = 16, 64, 4096
NCORES = 8
RPC = S * B // NCORES          # 512 rows per core
NP = H // 2                    # 8 head pairs
NKT = S // 128                 # 16 key tiles
GROUPS = [[0, 1, 2, 3], [4, 5, 6, 7]]

_TCNT = [0]


def _mk(pool, shape, dt, tag):
    _TCNT[0] += 1
    return pool.tile(shape, dt, tag=tag, name=f"t{_TCNT[0]}_{tag}")


def build_nc(n_rep=1, use_collective=True, stubs=frozenset()):
    nc = bacc.Bacc(num_devices=NCORES)

    xc_in = nc.dram_tensor("xc", [RPC, D], F32R, kind="ExternalInput")
    wq_in = nc.dram_tensor("wq", [8, 2, 128, 512], F32R, kind="ExternalInput")
    wk_in = nc.dram_tensor("wk", [8, 2, 128, 512], F32R, kind="ExternalInput")
    wv_in = nc.dram_tensor("wv", [8, 2, 128, 512], F32R, kind="ExternalInput")
    wo_in = nc.dram_tensor("wo", [8, 2, 128, 512], F32R, kind="ExternalInput")
    w1_in = nc.dram_tensor("w1", [8, 8, 128, 512], F32R, kind="ExternalInput")
    w2_in = nc.dram_tensor("w2", [8, 32, 128, 128], F32R, kind="ExternalInput")
    bq_in = nc.dram_tensor("bq", [D, 1], F32, kind="ExternalInput")
    bk_in = nc.dram_tensor("bk", [D, 1], F32, kind="ExternalInput")
    bv_in = nc.dram_tensor("bv", [D, 1], F32, kind="ExternalInput")
    bo_in = nc.dram_tensor("bo", [D, 1], F32, kind="ExternalInput")
    b1_in = nc.dram_tensor("b1", [DFF, 1], F32, kind="ExternalInput")
    b2_in = nc.dram_tensor("b2", [D, 1], F32, kind="ExternalInput")
    g1_in = nc.dram_tensor("g1", [1, D], F32, kind="ExternalInput")
    be1_in = nc.dram_tensor("be1", [1, D], F32, kind="ExternalInput")
    g2_in = nc.dram_tensor("g2", [1, D], F32, kind="ExternalInput")
    be2_in = nc.dram_tensor("be2", [1, D], F32, kind="ExternalInput")
    ident_in = nc.dram_tensor("ident", [128, 128], F32R, kind="ExternalInput")
    ones_in = nc.dram_tensor("ones1", [128, 64], F32R, kind="ExternalInput")
    out_d = nc.dram_tensor("out", [RPC, D], F32, kind="ExternalOutput")

    ag_bufs = []
    for rep in range(n_rep):
        ag_bufs.append((
            nc.dram_tensor(f"kT_ain{rep}", [D, RPC], F32R),
            nc.dram_tensor(f"kT_aout{rep}", [4 * D, RPC], F32R),
            nc.dram_tensor(f"v_ain{rep}", [RPC, D], F32R),
            nc.dram_tensor(f"v_aout{rep}", [4 * RPC, D], F32R),
        ))

    ins = dict(xc=xc_in, wq=wq_in, wk=wk_in, wv=wv_in, wo=wo_in,
               w1=w1_in, w2=w2_in, bq=bq_in, bk=bk_in, bv=bv_in, bo=bo_in,
               b1=b1_in, b2=b2_in, g1=g1_in, be1=be1_in, g2=g2_in,
               be2=be2_in, ident=ident_in, ones1=ones_in, out=out_d)

    with TileContext(nc) as tc:
        for rep in range(n_rep):
            _body(nc, tc, ins, ag_bufs[rep], use_collective, stubs)

    nc.finalize()
    return nc


def _body(nc, tc, ins, ag, use_collective, stubs=frozenset()):
    kT_ain, kT_aout, v_ain, v_aout = ag

    with (
        tc.tile_pool(name="outer", bufs=1) as po,
        tc.tile_pool(name="psum", bufs=8, space="PSUM") as pp,
    ):
        # ---- constants ----
        ident = _mk(po, [128, 128], F32R, "ident")
        nc.sync.dma_start(out=ident[:], in_=ins["ident"][:])
        ones64 = _mk(po, [128, 64], F32R, "ones")
        nc.sync.dma_start(out=ones64[:], in_=ins["ones1"][:])
        bias = {}
        for nm, n in (("bq", 8), ("bk", 8), ("bv", 8), ("bo", 8),
                      ("b1", 32), ("b2", 8)):
            t = _mk(po, [128, n], F32, "b_" + nm)
            nc.sync.dma_start(out=t[:],
                              in_=ins[nm].rearrange("(i p) o -> p (i o)", p=128))
            bias[nm] = t
        lnw = {}
        for nm in ("g1", "be1", "g2", "be2"):
            t = _mk(po, [128, D], F32, "ln_" + nm)
            nc.sync.dma_start(out=t[:], in_=ins[nm].broadcast_to([128, D]))
            lnw[nm] = t

        # ---- persistent activations ----
        x_nat = [_mk(po, [128, D], F32R, f"x{r}") for r in range(4)]
        for r in range(4):
            nc.sync.dma_start(out=x_nat[r][:],
                              in_=ins["xc"][r * 128:(r + 1) * 128, :])
        wshare = None
        if "w" in stubs:
            wshare = _mk(po, [128, 8 * 128], F32R, "wshare")
            nc.sync.dma_start(out=wshare[:].rearrange("p (a j) -> a p j", a=2), in_=ins["wv"][0])
        qT = [_mk(po, [128, RPC], F32R, f"qT{p}") for p in range(NP)]
        oT = [_mk(po, [128, RPC], F32R, f"oT{p}") for p in range(NP)]
        x1n = [_mk(po, [128, D], F32R, f"x1n{r}") for r in range(4)]

        # ================= phase 1: xT, Q/K/V projections, AllGather ====
        with tc.tile_pool(name="qkv", bufs=1) as pq:
            xT = [_mk(pq, [128, RPC], F32R, f"xT{dc}") for dc in range(8)]
            for dc in range(8):
                for r in range(4):
                    ps = _mk(pp, [128, 512], F32R, "ps")
                    nc.tensor.transpose(ps[:, 0:128],
                                        x_nat[r][:, dc * 128:(dc + 1) * 128],
                                        ident[:])
                    nc.vector.tensor_copy(xT[dc][:, r * 128:(r + 1) * 128],
                                          ps[:, 0:128])

            kT = [_mk(pq, [128, RPC], F32R, f"kT{p}") for p in range(NP)]
            v_nat = [_mk(pq, [128, D], F32R, f"v{r}") for r in range(4)]

            # Q/K: per half (4 head pairs), stream weight tiles [128,512]
            for nm, dst, b in (("wq", qT, "bq"), ("wk", kT, "bk")):
                for pg in range(2):
                    pss = {p: _mk(pp, [128, 512], F32, "ps")
                           for p in range(pg * 4, pg * 4 + 4)}
                    for dc in range(8):
                        wt = pq.tile([128, 512], F32R, tag="wsb", bufs=4,
                                     name=f"w_{nm}{pg}{dc}")
                        if "w" in stubs:
                            wt = wshare
                        else:
                            eng = nc.sync if dc % 2 == 0 else nc.scalar
                            eng.dma_start(out=wt[:], in_=ins[nm][dc, pg])
                        for j, p in enumerate(sorted(pss)):
                            nc.tensor.matmul(pss[p][:],
                                             wt[:, j * 128:(j + 1) * 128],
                                             xT[dc][:],
                                             start=(dc == 0), stop=(dc == 7))
                    for p in pss:
                        nc.vector.tensor_scalar(dst[p][:], pss[p][:],
                                                bias[b][:, p:p + 1], None,
                                                OP.add)

            # V: natural orientation, stream wv [128,512] halves
            for hf in range(2):
                pss = [_mk(pp, [128, 512], F32, "ps") for _ in range(4)]
                for dc in range(8):
                    wt = pq.tile([128, 512], F32R, tag="wsb", bufs=4,
                                 name=f"w_wv{hf}{dc}")
                    if "w" in stubs:
                        wt = wshare
                    else:
                        eng = nc.sync if dc % 2 == 0 else nc.scalar
                        eng.dma_start(out=wt[:], in_=ins["wv"][dc, hf])
                    for r in range(4):
                        nc.tensor.matmul(pss[r][:],
                                         xT[dc][:, r * 128:(r + 1) * 128],
                                         wt[:, 0:512],
                                         start=(dc == 0), stop=(dc == 7))
                for r in range(4):
                    nc.vector.tensor_copy(
                        v_nat[r][:, hf * 512:(hf + 1) * 512], pss[r][:])

            for p in range(NP):
                nc.sync.dma_start(out=kT_ain[p * 128:(p + 1) * 128, :],
                                  in_=kT[p][:])
            for r in range(4):
                nc.sync.dma_start(out=v_ain[r * 128:(r + 1) * 128, :],
                                  in_=v_nat[r][:])

        if "ag" in stubs:
            pass
        elif use_collective:
            nc.gpsimd.collective_compute("AllGather", OP.bypass,
                                         ins=[kT_ain[:]], outs=[kT_aout[:]],
                                         replica_groups=GROUPS)
            nc.gpsimd.collective_compute("AllGather", OP.bypass,
                                         ins=[v_ain[:]], outs=[v_aout[:]],
                                         replica_groups=GROUPS)
        else:
            for c in range(4):
                nc.sync.dma_start(out=kT_aout[c * D:(c + 1) * D, :],
                                  in_=kT_ain[:])
                nc.sync.dma_start(out=v_aout[c * RPC:(c + 1) * RPC, :],
                                  in_=v_ain[:])

        # bv in per-head layout: col h holds bv[h*64:(h+1)*64] at partitions 0-63
        bvh = _mk(po, [128, H], F32, "bvh")
        nc.sync.dma_start(out=bvh[0:64, :],
                          in_=ins["bv"].rearrange("(h p) o -> p (h o)", p=64))

        # ================= phase 2: attention =================
        # f32r matmuls reject column tiling, so each head's PV output lives
        # at psum partitions 0:65 (65th row = denominator via the ones
        # column appended to V). Head B is assembled into oT[64:128] by a
        # partition-shifting SBUF->SBUF DMA.
        with tc.tile_pool(name="attn", bufs=1) as pa:
            if "attn" in stubs:
                for p in range(NP):
                    nc.vector.tensor_copy(oT[p][:], qT[p][:])
            for p in range([], range(NP))["attn" not in stubs] if False else (range(0) if "attn" in stubs else range(NP)):
                kts = []
                for c in range(4):
                    t = pa.tile([128, RPC], F32R, tag="kts", bufs=8,
                                name=f"kts{p}_{c}")
                    nc.sync.dma_start(
                        out=t[:],
                        in_=kT_aout[c * D + p * 128:c * D + (p + 1) * 128, :])
                    kts.append(t)
                # vt: per key-tile 130 cols: [V_A(64) | ones] [V_B(64) | ones]
                vt = pa.tile([128, 16 * 130], F32R, tag="vt", bufs=2,
                             name=f"vt{p}")
                vt3 = vt[:].rearrange("p (c j) -> p c j", j=130)
                for hh in range(2):
                    nc.sync.dma_start(
                        out=vt3[:, :, hh * 65:hh * 65 + 64],
                        in_=v_aout[:, p * 128 + hh * 64:p * 128 + (hh + 1) * 64]
                            .rearrange("(c p) j -> p c j", p=128))
                    nc.sync.dma_start(
                        out=vt3[:, :, hh * 65 + 64:hh * 65 + 65],
                        in_=ins["ones1"][:, None, 0:1]
                            .broadcast_to([128, 16, 1]))

                ps_o = [_mk(pp, [128, 512], F32, "ps") for _ in range(2)]
                for kt in range(NKT):
                    c, ksub = divmod(kt, 4)
                    ps_s = [_mk(pp, [128, 512], F32, "ps") for _ in range(2)]
                    for hh in range(2):
                        nc.tensor.matmul(
                            ps_s[hh][:],
                            kts[c][hh * 64:(hh + 1) * 64,
                                   ksub * 128:(ksub + 1) * 128],
                            qT[p][hh * 64:(hh + 1) * 64, :],
                            start=True, stop=True, skip_group_check=True)
                    for hh in range(2):
                        pt_f = pa.tile([128, RPC], F32, tag="ptf", bufs=3,
                                       name=f"ptf{p}_{kt}_{hh}")
                        nc.scalar.activation(pt_f[:], ps_s[hh][:], AF.Exp,
                                             bias=0.0, scale=0.125)
                        pt_r = pa.tile([128, RPC], F32R, tag="ptr", bufs=3,
                                       name=f"ptr{p}_{kt}_{hh}")
                        nc.vector.tensor_copy(pt_r[:], pt_f[:])
                        nc.tensor.matmul(
                            ps_o[hh][0:65, :],
                            vt[:, kt * 130 + hh * 65:kt * 130 + (hh + 1) * 65],
                            pt_r[:], start=(kt == 0), stop=(kt == NKT - 1),
                            skip_group_check=True)
                # normalize per head: o = O[0:64] * bcast(1/den) + bv_head
                for hh in range(2):
                    h = 2 * p + hh
                    den_sb = pa.tile([128, RPC], F32, tag="den", bufs=2,
                                     name=f"den{p}_{hh}")
                    nc.scalar.copy(den_sb[64:65, :], ps_o[hh][64:65, :])
                    rden = pa.tile([128, RPC], F32R, tag="rden", bufs=2,
                                   name=f"rden{p}_{hh}")
                    with nc.allow_low_precision("f32r rounding of 1/den"):
                        nc.vector.reciprocal(rden[64:65, :], den_sb[64:65, :])
                    ps_b = _mk(pp, [128, 512], F32, "ps")
                    nc.tensor.matmul(ps_b[0:64, :], ones64[64:65, :],
                                     rden[64:65, :], start=True, stop=True,
                                     skip_group_check=True)
                    rb = pa.tile([128, RPC], F32, tag="rb", bufs=2,
                                 name=f"rb{p}_{hh}")
                    nc.scalar.copy(rb[0:64, :], ps_b[0:64, :])
                    if hh == 0:
                        tmp = pa.tile([128, RPC], F32, tag="onorm", bufs=2,
                                      name=f"onorm{p}_{hh}")
                        nc.vector.tensor_tensor(tmp[0:64, :], ps_o[hh][0:64, :],
                                                rb[0:64, :], OP.mult)
                        nc.vector.tensor_scalar(oT[p][0:64, :], tmp[0:64, :],
                                                bvh[0:64, h:h + 1], None,
                                                OP.add)
                    else:
                        stage = pa.tile([128, RPC], F32R, tag="stage", bufs=2,
                                        name=f"stage{p}")
                        tmp = pa.tile([128, RPC], F32, tag="onorm", bufs=2,
                                      name=f"onorm{p}_{hh}")
                        nc.vector.tensor_tensor(tmp[0:64, :], ps_o[hh][0:64, :],
                                                rb[0:64, :], OP.mult)
                        nc.vector.tensor_scalar(stage[0:64, :], tmp[0:64, :],
                                                bvh[0:64, h:h + 1], None,
                                                OP.add)
                        nc.sync.dma_start(out=oT[p][64:128, :],
                                          in_=stage[0:64, :])

        # ================= phase 3: Wo, LN1, FFN, LN2, out ==============
        with tc.tile_pool(name="post", bufs=1) as pf:
            x1 = [_mk(pf, [128, D], F32, f"x1_{r}") for r in range(4)]
            for ocg in range(2):
                pss = {oc: _mk(pp, [128, 512], F32, "ps")
                       for oc in range(ocg * 4, ocg * 4 + 4)}
                for dc in range(8):
                    wt = pf.tile([128, 512], F32R, tag="wosb", bufs=2,
                                 name=f"w_wo{ocg}{dc}")
                    if "w" in stubs:
                        wt = wshare
                    else:
                        eng = nc.sync if dc % 2 == 0 else nc.scalar
                        eng.dma_start(out=wt[:], in_=ins["wo"][dc, ocg])
                    for j, oc in enumerate(sorted(pss)):
                        nc.tensor.matmul(pss[oc][:],
                                         wt[:, j * 128:(j + 1) * 128],
                                         oT[dc][:],
                                         start=(dc == 0), stop=(dc == 7))
                for oc in pss:
                    mt = pf.tile([128, RPC], F32R, tag="mhaT", bufs=2,
                                 name=f"mhaT{oc}")
                    nc.vector.tensor_scalar(mt[:], pss[oc][:],
                                            bias["bo"][:, oc:oc + 1], None,
                                            OP.add)
                    for r in range(4):
                        ps = _mk(pp, [128, 512], F32R, "ps")
                        nc.tensor.transpose(ps[:, 0:128],
                                            mt[:, r * 128:(r + 1) * 128],
                                            ident[:])
                        nc.vector.tensor_tensor(
                            x1[r][:, oc * 128:(oc + 1) * 128],
                            ps[:, 0:128].bitcast(F32),
                            x_nat[r][:, oc * 128:(oc + 1) * 128].bitcast(F32),
                            OP.add)

            for r in range(4):
                _layernorm(nc, pf, x1n[r], x1[r], lnw["g1"], lnw["be1"])
            # x1nT reuses the oT tags (oT dead after the Wo matmuls)
            x1nT = [_mk(po, [128, RPC], F32R, f"oT{dc}") for dc in range(8)]
            for dc in range(8):
                for r in range(4):
                    ps = _mk(pp, [128, 512], F32R, "ps")
                    nc.tensor.transpose(ps[:, 0:128],
                                        x1n[r][:, dc * 128:(dc + 1) * 128],
                                        ident[:])
                    nc.vector.tensor_copy(x1nT[dc][:, r * 128:(r + 1) * 128],
                                          ps[:, 0:128])

            # FFN1: hT[g] holds 4 ffc slabs side by side [128, 4*512]
            hT = [_mk(pf, [128, 4 * RPC], F32R, f"hT{g}") for g in range(8)]
            for g in (range(0) if "ffn" in stubs else range(8)):
                pss = [_mk(pp, [128, 512], F32, "ps") for _ in range(4)]
                for dc in range(8):
                    w1t = pf.tile([128, 512], F32R, tag="w1t", bufs=2,
                                  name=f"w1t{g}_{dc}")
                    if "w" in stubs:
                        w1t = wshare
                    else:
                        eng = nc.sync if dc % 2 == 0 else nc.scalar
                        eng.dma_start(out=w1t[:], in_=ins["w1"][g, dc])
                    for j in range(4):
                        nc.tensor.matmul(pss[j][:],
                                         w1t[:, j * 128:(j + 1) * 128],
                                         x1nT[dc][:],
                                         start=(dc == 0), stop=(dc == 7))
                for j in range(4):
                    f = g * 4 + j
                    nc.scalar.activation(hT[g][:, j * RPC:(j + 1) * RPC],
                                         pss[j][:], AF.Relu,
                                         bias=bias["b1"][:, f:f + 1],
                                         scale=1.0)

            # FFN2 + transpose + residual
            x2 = [_mk(pf, [128, D], F32, f"x1_{r}") for r in range(4)]
            if "ffn" in stubs:
                for g in range(8):
                    for j in range(4):
                        nc.vector.tensor_copy(hT[g][:, j * RPC:(j + 1) * RPC],
                                              x1nT[j][:])
            for oc in range(8):
                ps2 = _mk(pp, [128, 512], F32, "ps")
                for qrt in range(4):
                    w2t = pf.tile([128, 8 * 128], F32R, tag="w2t", bufs=2,
                                  name=f"w2t{oc}_{qrt}")
                    if "w" in stubs:
                        w2t = wshare
                    else:
                        eng = nc.sync if qrt % 2 == 0 else nc.scalar
                        eng.dma_start(
                            out=w2t[:].rearrange("p (f j) -> p f j", f=8),
                            in_=ins["w2"][oc, qrt * 8:(qrt + 1) * 8]
                                .rearrange("f p j -> p f j"))
                    for fj in range(8):
                        f = qrt * 8 + fj
                        nc.tensor.matmul(
                            ps2[:], w2t[:, fj * 128:(fj + 1) * 128],
                            hT[f // 4][:, (f % 4) * RPC:(f % 4 + 1) * RPC],
                            start=(f == 0), stop=(f == 31))
                ft = pf.tile([128, RPC], F32R, tag="ffnT", bufs=2,
                             name=f"ffnT{oc}")
                nc.vector.tensor_scalar(ft[:], ps2[:], bias["b2"][:, oc:oc + 1],
                                        None, OP.add)
                for r in range(4):
                    ps = _mk(pp, [128, 512], F32R, "ps")
                    nc.tensor.transpose(ps[:, 0:128],
                                        ft[:, r * 128:(r + 1) * 128], ident[:])
                    nc.vector.tensor_tensor(
                        x2[r][:, oc * 128:(oc + 1) * 128],
                        ps[:, 0:128].bitcast(F32),
                        x1n[r][:, oc * 128:(oc + 1) * 128].bitcast(F32),
                        OP.add)

            # LN2 -> out (outt reuses the x tags; x dead after Wo residual)
            outt = [_mk(po, [128, D], F32, f"x{r}") for r in range(4)]
            for r in range(4):
                _layernorm(nc, pf, outt[r], x2[r], lnw["g2"], lnw["be2"])
                nc.sync.dma_start(out=ins["out"][r * 128:(r + 1) * 128, :],
                                  in_=outt[r][:])


def _layernorm(nc, pool, out, x, g, be):
    """LN along the free dim (D). x [128, 1024] f32; out f32 or f32r."""
    mu = pool.tile([128, 1], F32, tag="ln_mu", bufs=2, name=None)
    nc.vector.reduce_sum(mu[:], x[:], axis=AX.X)
    nc.vector.tensor_scalar_mul(mu[:], mu[:], 1.0 / D)
    t = pool.tile([128, D], F32, tag="ln_t", bufs=2, name=None)
    nc.vector.tensor_scalar(t[:], x[:], mu[:], None, OP.subtract)
    sq = pool.tile([128, 1], F32, tag="ln_sq", bufs=2, name=None)
    sq2 = pool.tile([128, D], F32, tag="ln_sq2", bufs=2, name=None)
    nc.scalar.activation(sq2[:], t[:], AF.Square, bias=0.0, scale=1.0,
                         accum_out=sq[:])
    var = pool.tile([128, 1], F32, tag="ln_var", bufs=2, name=None)
    nc.vector.tensor_scalar(var[:], sq[:], 1.0 / D, 1e-5, OP.mult, OP.add)
    std = pool.tile([128, 1], F32, tag="ln_std", bufs=2, name=None)
    nc.scalar.sqrt(std[:], var[:])
    rstd = pool.tile([128, 1], F32, tag="ln_rstd", bufs=2, name=None)
    nc.vector.reciprocal(rstd[:], std[:])
    t2 = pool.tile([128, D], F32, tag="ln_sq2", bufs=2, name=None)
    nc.vector.tensor_scalar_mul(t2[:], t[:], rstd[:])
    t3 = pool.tile([128, D], F32, tag="ln_t", bufs=2, name=None)
    nc.vector.tensor_tensor(t3[:], t2[:], g[:], OP.mult)
    nc.vector.tensor_tensor(out[:], t3[:], be[:], OP.add)


_LN_CNT = [0]
_orig_ln = _layernorm


def _layernorm(nc, pool, out, x, g, be, _orig=_orig_ln):  # noqa: F811
    # wrap to generate unique tile names (pool.tile needs explicit names)
    _LN_CNT[0] += 1
    n = _LN_CNT[0]

    class _P:
        def tile(self, shape, dt, tag, bufs, name):
            _TCNT[0] += 1
            return pool.tile(shape, dt, tag=tag, bufs=bufs,
                             name=f"ln{n}_{tag}_{_TCNT[0]}")

    return _orig(nc, _P(), out, x, g, be)


def prep_inputs(x, Wq, bq, Wk, bk, Wv, bv, Wo, bo, W1, b1, W2, b2,
                g1, be1, g2, be2):
    """Host-side prep: per-core shards + kernel weight layouts (all f32)."""
    f = np.float32
    def _dc_pg(w):  # [D, D] -> [dc, pg, 128, 512]
        return np.ascontiguousarray(
            np.asarray(w, f).reshape(8, 128, 2, 512).transpose(0, 2, 1, 3))
    wq2 = _dc_pg(np.asarray(Wq, f).transpose(1, 0, 2).reshape(D, D))
    wk2 = _dc_pg(np.asarray(Wk, f).transpose(1, 0, 2).reshape(D, D))
    wv2 = _dc_pg(np.asarray(Wv, f).transpose(1, 0, 2).reshape(D, D))
    wo2 = _dc_pg(np.asarray(Wo, f))
    w12 = np.ascontiguousarray(
        np.asarray(W1, f).reshape(8, 128, 8, 512).transpose(2, 0, 1, 3))
    w22 = np.ascontiguousarray(
        np.asarray(W2, f).reshape(32, 128, 8, 128).transpose(2, 0, 1, 3))
    common = {
        "wq": wq2, "wk": wk2, "wv": wv2, "wo": wo2, "w1": w12, "w2": w22,
        "bq": np.asarray(bq, f).reshape(D, 1),
        "bk": np.asarray(bk, f).reshape(D, 1),
        "bv": np.asarray(bv, f).reshape(D, 1),
        "bo": np.asarray(bo, f).reshape(D, 1),
        "b1": np.asarray(b1, f).reshape(DFF, 1),
        "b2": np.asarray(b2, f).reshape(D, 1),
        "g1": np.asarray(g1, f).reshape(1, D),
        "be1": np.asarray(be1, f).reshape(1, D),
        "g2": np.asarray(g2, f).reshape(1, D),
        "be2": np.asarray(be2, f).reshape(1, D),
        "ident": np.eye(128, dtype=f),
        "ones1": np.ones((128, 64), dtype=f),
    }
    xf = np.asarray(x, f)
    in_maps = []
    for c in range(NCORES):
        b, j = divmod(c, 4)
        m = dict(common)
        m["xc"] = np.ascontiguousarray(xf[b, j * RPC:(j + 1) * RPC, :])
        in_maps.append(m)
    return in_maps


_NC_CACHE = {}
LAST_EXEC_NS = None
LAST_TRACE_PATH = None
LAST_PROFILE_JSON = None


def kernel(**inputs) -> np.ndarray:
    global LAST_EXEC_NS, LAST_TRACE_PATH, LAST_PROFILE_JSON
    if "main" not in _NC_CACHE:
        _NC_CACHE["main"] = build_nc(n_rep=1, use_collective=True)
    nc = _NC_CACHE["main"]
    in_maps = prep_inputs(**inputs)
    res = run_bass_kernel_spmd(nc, in_maps, core_ids=list(range(NCORES)))
    LAST_EXEC_NS = getattr(res, "exec_time_ns", None)
    LAST_PROFILE_JSON = getattr(res, "profile_json", None)
    it = getattr(res, "instructions_and_trace", None)
    LAST_TRACE_PATH = it[1] if it else None
    out = np.empty((B, S, D), np.float32)
    for c in range(NCORES):
        b, j = divmod(c, 4)
        out[b, j * RPC:(j + 1) * RPC, :] = res.results[c]["out"]
    return out



# revision 11
# speedup vs baseline: 1.6736x; 1.6736x over previous
"""Trainium2 Bass kernel for nn_Encoder (dense transformer encoder layer).

Sharding: 8 NeuronCores, sequence-parallel, zero collectives. B*S = 4096
rows -> 512 rows per core; cores 0-3 handle batch 0, cores 4-7 batch 1.
The full batch's x (transposed, bf16) is replicated to every core as an
input, so each core computes K^T and V for the WHOLE batch locally (no
K/V AllGather), then runs attention for its own 512 query rows over all
16 heads, plus Wo / LN1 / FFN / LN2 locally.

All matmuls bf16 (full PE rate, weights pre-cast host-side); psum
accumulation fp32. End-to-end relative error ~3e-3 (budget 2e-2).

Schedule: K/V/Q projections for head-pair p+2 are woven between the
attention units of pair p (thunk queue), so the PE stays dense while
the scalar engine streams the softmax exp() ops.

Dataflow (feature-on-partition):
  xT [8dc x 128, 2048] bf16 (host-transposed full-batch input)
  kT[p] [128, 2048] = Wk_p.T @ xT  (full batch keys)
  vq[q]: V columns for pairs 2q,2q+1, keys on partition, ones column
         appended per (pair,hh,keytile) for the softmax denominator
  qT[p] [128, 512] from xoT (own rows, host-transposed)
  S^T [128 keys, 512 q] = kts.T @ qT  (2 heads packed at rows 0/64)
  P = exp(0.125 * S^T) -> bf16, one ACT op per 2 key tiles
  O^T[65,512] += [V|1].T @ P  (psum row 64 = denominator)
  oT = O^T * bcast(1/den) + bv;  mhaT = Wo.T @ oT (+bo)
  x1 = transpose(mhaT) + x -> LN1 -> x1n -> x1nT
  hT = relu(W1.T @ x1nT + b1); ffnT = W2.T @ hT
  x2T = ffnT + b2 + x1nT -> transpose -> LN2 -> out [512, 1024] f32
"""

from collections import deque

import numpy as np

import concourse.bass as bass
import concourse.mybir as mybir
from concourse import bacc
from concourse.tile import TileContext
from concourse.bass_utils import run_bass_kernel_spmd

F32 = mybir.dt.float32
BF = mybir.dt.bfloat16
AF = mybir.ActivationFunctionType
OP = mybir.AluOpType

B, S, D = 2, 2048, 1024
H, DK, DFF = 16, 64, 4096
NCORES = 8
RPC = S * B // NCORES          # 512 own rows per core
FB = S                         # 2048 full-batch rows
NP = H // 2                    # 8 head pairs
NKT = FB // 128                # 16 key tiles

_TCNT = [0]


def _mk(pool, shape, dt, tag, bufs=None):
    _TCNT[0] += 1
    kw = {} if bufs is None else {"bufs": bufs}
    return pool.tile(shape, dt, tag=tag, name=f"t{_TCNT[0]}_{tag}", **kw)


def build_nc():
    nc = bacc.Bacc(num_devices=NCORES)

    ins = dict(
        xbT=nc.dram_tensor("xbT", [D, FB], BF, kind="ExternalInput"),
        xoT=nc.dram_tensor("xoT", [D, RPC], BF, kind="ExternalInput"),
        xn=nc.dram_tensor("xn", [RPC, D], BF, kind="ExternalInput"),
        wq=nc.dram_tensor("wq", [NP, 128, 1024], BF, kind="ExternalInput"),
        wk=nc.dram_tensor("wk", [NP, 128, 1024], BF, kind="ExternalInput"),
        wv=nc.dram_tensor("wv", [4, 128, 2048], BF, kind="ExternalInput"),
        wo=nc.dram_tensor("wo", [8, 128, 1024], BF, kind="ExternalInput"),
        w1=nc.dram_tensor("w1", [32, 128, 1024], BF, kind="ExternalInput"),
        w2=nc.dram_tensor("w2", [32, 128, 1024], BF, kind="ExternalInput"),
        bq=nc.dram_tensor("bq", [D, 1], F32, kind="ExternalInput"),
        bk=nc.dram_tensor("bk", [D, 1], F32, kind="ExternalInput"),
        bv=nc.dram_tensor("bv", [D, 1], F32, kind="ExternalInput"),
        bvr=nc.dram_tensor("bvr", [1, D], BF, kind="ExternalInput"),
        bo=nc.dram_tensor("bo", [D, 1], F32, kind="ExternalInput"),
        b1=nc.dram_tensor("b1", [DFF, 1], F32, kind="ExternalInput"),
        b2=nc.dram_tensor("b2", [D, 1], F32, kind="ExternalInput"),
        g1=nc.dram_tensor("g1", [1, D], BF, kind="ExternalInput"),
        be1=nc.dram_tensor("be1", [1, D], BF, kind="ExternalInput"),
        g2=nc.dram_tensor("g2", [1, D], BF, kind="ExternalInput"),
        be2=nc.dram_tensor("be2", [1, D], BF, kind="ExternalInput"),
        ident=nc.dram_tensor("ident", [128, 128], BF, kind="ExternalInput"),
        ones1=nc.dram_tensor("ones1", [128, 64], BF, kind="ExternalInput"),
        out=nc.dram_tensor("out", [RPC, D], F32, kind="ExternalOutput"),
    )

    with TileContext(nc) as tc:
        _body(nc, tc, ins)

    nc.finalize()
    return nc


def _body(nc, tc, ins):
    with (
        tc.tile_pool(name="outer", bufs=1) as po,
        tc.tile_pool(name="psum", bufs=1, space="PSUM") as pp,
    ):
        # ---- constants ----
        ident = _mk(po, [128, 128], BF, "ident")
        nc.scalar.dma_start(out=ident[:], in_=ins["ident"][:])
        ones = _mk(po, [128, 64], BF, "ones")
        nc.scalar.dma_start(out=ones[:], in_=ins["ones1"][:])
        bias = {}
        for nm, n in (("bq", 8), ("bk", 8), ("bo", 8), ("b1", 32), ("b2", 8)):
            t = _mk(po, [128, n], F32, "b_" + nm)
            nc.scalar.dma_start(out=t[:],
                                in_=ins[nm].rearrange("(i p) o -> p (i o)", p=128))
            bias[nm] = t
        # bv in per-head layout: col h = bv[h*64:(h+1)*64] on partitions 0-63
        bvh = _mk(po, [128, H], F32, "bvh")
        nc.scalar.dma_start(out=bvh[0:64, :],
                            in_=ins["bv"].rearrange("(h p) o -> p (h o)", p=64))
        bvr = _mk(po, [128, D], BF, "bvr")
        nc.scalar.dma_start(out=bvr[:], in_=ins["bvr"].broadcast_to([128, D]))
        lnw = {}
        for nm in ("g1", "be1", "g2", "be2"):
            t = _mk(po, [128, D], BF, "ln_" + nm)
            nc.scalar.dma_start(out=t[:], in_=ins[nm].broadcast_to([128, D]))
            lnw[nm] = t
        eps = _mk(po, [128, 1], F32, "eps")
        nc.vector.memset(eps[:], 1e-5)
        lnw["eps"] = eps
        xn = [_mk(po, [128, D], BF, f"xn{r}") for r in range(4)]
        for r in range(4):
            nc.scalar.dma_start(out=xn[r][:],
                                in_=ins["xn"][r * 128:(r + 1) * 128, :])

        # persistent post-phase activations
        mhaT = [_mk(po, [128, RPC], BF, f"mhaT{oc}") for oc in range(8)]
        x1 = [_mk(po, [128, D], BF, f"x1_{r}") for r in range(4)]
        x1n = [_mk(po, [128, D], BF, f"x1n{r}") for r in range(4)]
        x1nT = [_mk(po, [128, RPC], BF, f"x1nT{dc}") for dc in range(8)]

        with tc.tile_pool(name="attn", bufs=1) as pa:
            _attn_phase(nc, tc, ins, po, pa, pp, ident, ones, bias, bvh, bvr,
                        xn, mhaT, x1)

        with tc.tile_pool(name="post", bufs=1) as pf:
            _post_phase(nc, tc, ins, pf, pp, ident, bias, lnw,
                        x1, x1n, x1nT)


def _attn_phase(nc, tc, ins, po, pa, pp, ident, ones, bias, bvh, bvr,
                xn, mhaT, x1):
    oT = [_mk(pa, [128, RPC], BF, f"oT{p}") for p in range(NP)]
    kTs, qTs, vqs = {}, {}, {}

    with tc.tile_pool(name="proj", bufs=1) as px:
        xT = [_mk(px, [128, FB], BF, f"xT{dc}") for dc in range(8)]
        for dc in range(8):
            nc.sync.dma_start(out=xT[dc][:],
                              in_=ins["xbT"][dc * 128:(dc + 1) * 128, :])
        xoT = [_mk(px, [128, RPC], BF, f"xoT{dc}") for dc in range(8)]
        for dc in range(8):
            nc.scalar.dma_start(out=xoT[dc][:],
                                in_=ins["xoT"][dc * 128:(dc + 1) * 128, :])

        def proj_thunks(p):
            """Thunk list computing kT/qT (and V quarter when p even)."""
            thunks = []
            st = {}

            def dma_w():
                wkt = _mk(px, [128, 1024], BF, "wkt", bufs=3)
                nc.sync.dma_start(out=wkt[:], in_=ins["wk"][p])
                wqt = _mk(px, [128, 1024], BF, "wqt", bufs=3)
                nc.sync.dma_start(out=wqt[:], in_=ins["wq"][p])
                kTs[p] = _mk(pa, [128, FB], BF, "kT", bufs=3)
                qTs[p] = _mk(pa, [128, RPC], BF, "qT", bufs=3)
                st["wkt"], st["wqt"] = wkt, wqt

            thunks.append(dma_w)

            def k_chunk(c):
                ps = _mk(pp, [128, 512], F32, "ps_p", bufs=2)
                for dc in range(8):
                    nc.tensor.matmul(ps[:],
                                     st["wkt"][:, dc * 128:(dc + 1) * 128],
                                     xT[dc][:, c * 512:(c + 1) * 512],
                                     start=(dc == 0), stop=(dc == 7),
                                     skip_group_check=True)
                nc.vector.tensor_scalar(kTs[p][:, c * 512:(c + 1) * 512],
                                        ps[:], bias["bk"][:, p:p + 1],
                                        None, OP.add)

            for c in range(4):
                thunks.append(lambda c=c: k_chunk(c))

            def q_chunk():
                ps = _mk(pp, [128, 512], F32, "ps_p", bufs=2)
                for dc in range(8):
                    nc.tensor.matmul(ps[:],
                                     st["wqt"][:, dc * 128:(dc + 1) * 128],
                                     xoT[dc][:], start=(dc == 0), stop=(dc == 7),
                                     skip_group_check=True)
                nc.vector.tensor_scalar(qTs[p][:], ps[:],
                                        bias["bq"][:, p:p + 1], None, OP.add)

            thunks.append(q_chunk)

            if p % 2 == 0:
                q = p // 2

                def dma_v():
                    wvt = _mk(px, [128, 2048], BF, "wvt", bufs=2)
                    nc.sync.dma_start(out=wvt[:], in_=ins["wv"][q])
                    vq = _mk(pa, [128, 2 * 2 * NKT * 65], BF, "vq", bufs=2)
                    vqs[q] = vq
                    vqv = vq[:].rearrange("k (i h t c) -> k i h t c",
                                          i=2, h=2, c=65)
                    for pl in range(2):
                        nc.vector.memset(vqv[:, pl, :, :, 64:65], 1.0)
                    st["wvt"] = wvt

                thunks.append(dma_v)

                def v_chunk(kt2):
                    vqv = vqs[q][:].rearrange("k (i h t c) -> k i h t c",
                                              i=2, h=2, c=65)
                    bvs = bvr[:, q * 256:(q + 1) * 256].rearrange(
                        "k (i h c) -> k i h c", i=2, c=64)
                    ps = _mk(pp, [128, 512], F32, "ps_p", bufs=2)
                    for dc in range(8):
                        nc.tensor.matmul(ps[:, 0:256],
                                         xT[dc][:, kt2 * 128:(kt2 + 1) * 128],
                                         st["wvt"][:, dc * 256:(dc + 1) * 256],
                                         start=(dc == 0), stop=(dc == 7),
                                         skip_group_check=True)
                    nc.vector.tensor_tensor(
                        vqv[:, :, :, kt2, 0:64],
                        ps[:, 0:256].rearrange("k (i h c) -> k i h c",
                                               i=2, c=64),
                        bvs, OP.add)

                for kt2 in range(NKT):
                    thunks.append(lambda kt2=kt2: v_chunk(kt2))
            return thunks

        def attn_unit(p, u):
            kt, qt, vq = kTs[p], qTs[p], vqs[p // 2]
            pl = p % 2
            vqv = vq[:].rearrange("k (i h t c) -> k i h t c", i=2, h=2, c=65)
            pss = [_mk(pp, [128, 1024], F32, "ps_s", bufs=2) for _ in range(2)]
            for i in range(2):
                ktile = 2 * u + i
                for hh in range(2):
                    nc.tensor.matmul(
                        pss[hh][:, i * 512:(i + 1) * 512],
                        kt[hh * 64:(hh + 1) * 64,
                           ktile * 128:(ktile + 1) * 128],
                        qt[hh * 64:(hh + 1) * 64, :],
                        start=True, stop=True, skip_group_check=True)
            pts = []
            for hh in range(2):
                pt = _mk(pa, [128, 1024], BF, "pt", bufs=4)
                nc.scalar.activation(pt[:], pss[hh][:], AF.Exp,
                                     bias=0.0, scale=0.125)
                pts.append(pt)
            for i in range(2):
                ktile = 2 * u + i
                for hh in range(2):
                    nc.tensor.matmul(
                        _PSO[hh][0:65, :],
                        vqv[:, pl, hh, ktile, :],
                        pts[hh][:, i * 512:(i + 1) * 512],
                        start=(ktile == 0), stop=(ktile == NKT - 1),
                        skip_group_check=True)

        def attn_norm(p):
            for hh in range(2):
                h = 2 * p + hh
                den = _mk(pa, [128, 512], F32, "den", bufs=2)
                nc.vector.tensor_copy(den[64:65, :], _PSO[hh][64:65, :])
                rden = _mk(pa, [128, 512], BF, "rden", bufs=2)
                with nc.allow_low_precision("softmax 1/den in bf16"):
                    nc.vector.reciprocal(rden[64:65, :], den[64:65, :])
                ps_b = _mk(pp, [128, 512], F32, "ps_p", bufs=2)
                nc.tensor.matmul(ps_b[0:64, :], ones[64:65, :],
                                 rden[64:65, :], start=True, stop=True,
                                 skip_group_check=True)
                rb = _mk(pa, [128, 512], F32, "rb", bufs=2)
                nc.vector.tensor_copy(rb[0:64, :], ps_b[0:64, :])
                tmp = _mk(pa, [128, 512], F32, "onorm", bufs=2)
                nc.vector.tensor_tensor(tmp[0:64, :], _PSO[hh][0:64, :],
                                        rb[0:64, :], OP.mult)
                if hh == 0:
                    nc.vector.tensor_scalar(oT[p][0:64, :], tmp[0:64, :],
                                            bvh[0:64, h:h + 1], None, OP.add)
                else:
                    stage = _mk(pa, [128, 512], BF, "stage", bufs=2)
                    nc.vector.tensor_scalar(stage[0:64, :], tmp[0:64, :],
                                            bvh[0:64, h:h + 1], None, OP.add)
                    nc.gpsimd.dma_start(out=oT[p][64:128, :],
                                        in_=stage[0:64, :])

        # ---- software-pipelined schedule: proj runs 2 pairs ahead ----
        pending = deque()
        for t in proj_thunks(0) + proj_thunks(1):
            t()
        for p in range(NP):
            if p + 2 < NP:
                pending.extend(proj_thunks(p + 2))
            _PSO = [_mk(pp, [128, 512], F32, "ps_o", bufs=2) for _ in range(2)]
            for u in range(NKT // 2):
                attn_unit(p, u)
                slots_left = NKT // 2 - u
                k = (len(pending) + slots_left - 1) // slots_left
                for _ in range(min(k, len(pending))):
                    pending.popleft()()
            attn_norm(p)
        assert not pending

    # ---- Wo + residual (back to natural domain) ----
    for oc in range(8):
        wot = _mk(pa, [128, 1024], BF, "wot", bufs=4)
        nc.sync.dma_start(out=wot[:], in_=ins["wo"][oc])
        ps = _mk(pp, [128, 512], F32, "ps_p", bufs=2)
        for dc in range(8):
            nc.tensor.matmul(ps[:], wot[:, dc * 128:(dc + 1) * 128],
                             oT[dc][:], start=(dc == 0), stop=(dc == 7),
                             skip_group_check=True)
        nc.vector.tensor_scalar(mhaT[oc][:], ps[:],
                                bias["bo"][:, oc:oc + 1], None, OP.add)
    for r in range(4):
        ps = _mk(pp, [128, 1024], BF, "ps_p", bufs=2)
        for oc in range(8):
            nc.tensor.transpose(ps[:, oc * 128:(oc + 1) * 128],
                                mhaT[oc][:, r * 128:(r + 1) * 128], ident[:])
        nc.vector.tensor_tensor(x1[r][:], ps[:], xn[r][:], OP.add)


def _post_phase(nc, tc, ins, pf, pp, ident, bias, lnw, x1, x1n, x1nT):
    for r in range(4):
        _layernorm(nc, pf, x1n[r], x1[r], lnw["g1"], lnw["be1"], lnw["eps"])
    for d2 in range(4):
        ps = _mk(pp, [128, 1024], BF, "ps_p", bufs=2)
        for dl in range(2):
            dc = 2 * d2 + dl
            for r in range(4):
                nc.tensor.transpose(ps[:, dl * 512 + r * 128:
                                       dl * 512 + (r + 1) * 128],
                                    x1n[r][:, dc * 128:(dc + 1) * 128],
                                    ident[:])
        for dl in range(2):
            nc.vector.tensor_copy(x1nT[2 * d2 + dl][:],
                                  ps[:, dl * 512:(dl + 1) * 512])

    # ---- FFN ----
    w2t = [_mk(pf, [128, 1024], BF, f"w2t{f}") for f in range(32)]
    hT = [_mk(pf, [128, RPC], BF, f"hT{f}") for f in range(32)]
    for f in range(32):
        w1t = _mk(pf, [128, 1024], BF, "w1t", bufs=4)
        nc.sync.dma_start(out=w1t[:], in_=ins["w1"][f])
        nc.sync.dma_start(out=w2t[f][:], in_=ins["w2"][f])
        ps = _mk(pp, [128, 512], F32, "ps_p", bufs=2)
        for dc in range(8):
            nc.tensor.matmul(ps[:], w1t[:, dc * 128:(dc + 1) * 128],
                             x1nT[dc][:], start=(dc == 0), stop=(dc == 7),
                             skip_group_check=True)
        nc.scalar.activation(hT[f][:], ps[:], AF.Relu,
                             bias=bias["b1"][:, f:f + 1], scale=1.0)

    x2T = [_mk(pf, [128, RPC], BF, f"x2T{oc}") for oc in range(8)]
    for oc in range(8):
        ps = _mk(pp, [128, 512], F32, "ps_p", bufs=2)
        for f in range(32):
            nc.tensor.matmul(ps[:], w2t[f][:, oc * 128:(oc + 1) * 128],
                             hT[f][:], start=(f == 0), stop=(f == 31),
                             skip_group_check=True)
        tmp = _mk(pf, [128, RPC], BF, "f2tmp", bufs=2)
        nc.vector.tensor_tensor(tmp[:], ps[:], x1nT[oc][:], OP.add)
        nc.vector.tensor_scalar(x2T[oc][:], tmp[:],
                                bias["b2"][:, oc:oc + 1], None, OP.add)

    # ---- transpose back, LN2, out ----
    for r in range(4):
        ps = _mk(pp, [128, 1024], BF, "ps_p", bufs=2)
        for oc in range(8):
            nc.tensor.transpose(ps[:, oc * 128:(oc + 1) * 128],
                                x2T[oc][:, r * 128:(r + 1) * 128], ident[:])
        x2 = _mk(pf, [128, D], BF, "x2", bufs=2)
        nc.vector.tensor_copy(x2[:], ps[:])
        outt = _mk(pf, [128, D], F32, "outt", bufs=2)
        _layernorm(nc, pf, outt, x2, lnw["g2"], lnw["be2"], lnw["eps"])
        nc.sync.dma_start(out=ins["out"][r * 128:(r + 1) * 128, :],
                          in_=outt[:])


def _layernorm(nc, pool, out, x, g, be, eps):
    """LN along the free dim (D=1024). x [128, 1024] bf16; out bf16/f32."""
    _TCNT[0] += 1
    n = _TCNT[0]
    stats = pool.tile([128, 2, 6], F32, tag="ln_st", bufs=2, name=f"lnst{n}")
    for i in range(2):
        nc.vector.bn_stats(stats[:, i, :], x[:, i * 512:(i + 1) * 512])
    mv = pool.tile([128, 2], F32, tag="ln_mv", bufs=2, name=f"lnmv{n}")
    nc.vector.bn_aggr(mv[:], stats[:])
    std = pool.tile([128, 1], F32, tag="ln_sd", bufs=2, name=f"lnsd{n}")
    nc.scalar.activation(std[:], mv[:, 1:2], AF.Sqrt, bias=eps, scale=1.0)
    rstd = pool.tile([128, 1], F32, tag="ln_rs", bufs=2, name=f"lnrs{n}")
    nc.vector.reciprocal(rstd[:], std[:])
    t = pool.tile([128, D], BF, tag="ln_t", bufs=2, name=f"lnt{n}")
    nc.vector.tensor_scalar(t[:], x[:], mv[:, 0:1], rstd[:],
                            OP.subtract, OP.mult)
    t2 = pool.tile([128, D], BF, tag="ln_t2", bufs=2, name=f"lnt2{n}")
    nc.vector.tensor_tensor(t2[:], t[:], g[:], OP.mult)
    nc.vector.tensor_tensor(out[:], t2[:], be[:], OP.add)


def prep_inputs(x, Wq, bq, Wk, bk, Wv, bv, Wo, bo, W1, b1, W2, b2,
                g1, be1, g2, be2):
    """Host-side prep: per-core inputs, weights pre-cast to bf16.

    Stationary-weight layouts are [*, 128, n] with the 128 SBUF
    partitions contiguous-major so each tile is one dense DMA.
    """
    import ml_dtypes
    f = np.float32
    bf = ml_dtypes.bfloat16

    def _qdc(w, ncol):  # [D_in, ncols] -> [ncols/ncol, 128, 8*ncol]
        # element (blk, q, dc*ncol+c) = w[dc*128+q, blk*ncol+c]
        nblk = w.shape[1] // ncol
        return np.ascontiguousarray(
            np.asarray(w, f).reshape(8, 128, nblk, ncol).transpose(2, 1, 0, 3)
            .reshape(nblk, 128, 8 * ncol)).astype(bf)

    wq_flat = np.asarray(Wq, f).transpose(1, 0, 2).reshape(D, D)
    wk_flat = np.asarray(Wk, f).transpose(1, 0, 2).reshape(D, D)
    wv_flat = np.asarray(Wv, f).transpose(1, 0, 2).reshape(D, D)
    common = {
        "wq": _qdc(wq_flat, 128), "wk": _qdc(wk_flat, 128),
        "wv": _qdc(wv_flat, 256), "wo": _qdc(np.asarray(Wo, f), 128),
        "w1": _qdc(np.asarray(W1, f), 128),
        "w2": np.asarray(W2, f).reshape(32, 128, 1024).astype(bf),
        "bq": np.asarray(bq, f).reshape(D, 1),
        "bk": np.asarray(bk, f).reshape(D, 1),
        "bv": np.asarray(bv, f).reshape(D, 1),
        "bvr": np.asarray(bv, f).reshape(1, D).astype(bf),
        "bo": np.asarray(bo, f).reshape(D, 1),
        "b1": np.asarray(b1, f).reshape(DFF, 1),
        "b2": np.asarray(b2, f).reshape(D, 1),
        "g1": np.asarray(g1, f).reshape(1, D).astype(bf),
        "be1": np.asarray(be1, f).reshape(1, D).astype(bf),
        "g2": np.asarray(g2, f).reshape(1, D).astype(bf),
        "be2": np.asarray(be2, f).reshape(1, D).astype(bf),
        "ident": np.eye(128, dtype=f).astype(bf),
        "ones1": np.ones((128, 64), dtype=f).astype(bf),
    }
    xf = np.asarray(x, f)
    xbT = [np.ascontiguousarray(xf[b].T).astype(bf) for b in range(B)]
    in_maps = []
    for c in range(NCORES):
        b, j = divmod(c, 4)
        m = dict(common)
        m["xbT"] = xbT[b]
        own = xf[b, j * RPC:(j + 1) * RPC, :]
        m["xoT"] = np.ascontiguousarray(own.T).astype(bf)
        m["xn"] = np.ascontiguousarray(own).astype(bf)
        in_maps.append(m)
    return in_maps


_NC_CACHE = {}
LAST_EXEC_NS = None
LAST_TRACE_PATH = None
LAST_PROFILE_JSON = None


def kernel(**inputs) -> np.ndarray:
    global LAST_EXEC_NS, LAST_TRACE_PATH, LAST_PROFILE_JSON
    if "main" not in _NC_CACHE:
        _NC_CACHE["main"] = build_nc()
    nc = _NC_CACHE["main"]
    in_maps = prep_inputs(**inputs)
    res = run_bass_kernel_spmd(nc, in_maps, core_ids=list(range(NCORES)))
    LAST_EXEC_NS = getattr(res, "exec_time_ns", None)
    LAST_PROFILE_JSON = getattr(res, "profile_json", None)
    it = getattr(res, "instructions_and_trace", None)
    LAST_TRACE_PATH = it[1] if it else None
    out = np.empty((B, S, D), np.float32)
    for c in range(NCORES):
        b, j = divmod(c, 4)
        out[b, j * RPC:(j + 1) * RPC, :] = res.results[c]["out"]
    return out
